# revision 46
# baseline (speedup 1.0000x reference)
"""Trainium2 Bass kernel for nn_Attention_31147102831130.

Math (per token): qkv = x@Wqkv+b; per-position attention over the HEADS axis:
  q,k,v: [H=16, Dh=64]; A = softmax(q k^T / sqrt(1024)); o = A v (flat 1024)
  y = o@Wo + bo.

Sharding: pure data-parallel over batch: 8 cores x 2 batches (2048 tokens).

Per-core: 16 tiles of 128 tokens, processed as a lag-3 modulo-scheduled
pipeline so every engine runs back-to-back on work from different tiles:
  wave w:  PE qkv-proj(w) | Pool+DVE scores(w-1) | Pool+DVE A*V(w-2)
           | 1/Z + extraction + PE transposes + o-proj (w-3)
Engine roles:
  PE    : QKV projection (bf16), o transposes, output projection
  DVE   : score products + d-tree, A*V products + t-tree for most heads;
          reciprocal of the softmax denominator
  Pool  : same products/trees for the first PHPAT[i] heads of tile i
          (ramped 1,2,3,... over the fill tiles) plus one extra score
          product (XPROD) - balances DVE within ~5%
  ACT   : PSUM->SBUF copies, exp, v^T copy, per-head o_c extraction
          scaled by 1/Z (A*V consumes unnormalized bf16 exp scores)
Cross-engine edges all carry >=1 wave of slack so the in-order engine
queues never stall on a late producer. Product/tree tiles use single-buffer
pools with per-tensor tags so WAR hazards chain same-tile ops, which keeps
the greedy tile scheduler from interleaving two tiles' reduction trees.
All weights SBUF-resident; x streamed per-tile. Host pre-permutes Wqkv
columns to [Q|K|V] head-major and pre-transposes x so phase 1 needs no
on-device transposes.
"""

import os
import numpy as np
import ml_dtypes

B, S, C = 16, 1024, 1024
E, H, DH = 1024, 16, 64
NCORES = 8
TOK = B * S // NCORES      # 2048 tokens per core
PT = 128                   # tokens per tile
NT = TOK // PT             # 16 tiles
KC = C // 128              # 8 contraction chunks

_CACHE = {}
_PHSC = int(os.environ.get("KERNEL_PHSC", "3"))   # heads on Pool pipeline
_PHAV = _PHSC
_FLPOOL = os.environ.get("KERNEL_FL", "0") == "1"  # 1x final tree levels -> Pool
_LAG_T = 3


def _build_nc():
    import concourse.bass as bass
    import concourse.mybir as mybir
    from concourse.tile import TileContext
    from concourse.masks import make_identity
    from concourse.bass import ts, ds

    bf16 = mybir.dt.bfloat16
    f32 = mybir.dt.float32
    AF = mybir.ActivationFunctionType

    nc = bass.Bass()

    xT_d = nc.declare_dram_parameter("xT", [KC, 128, TOK], bf16, isOutput=False)
    wqkv_d = nc.declare_dram_parameter("wqkv", [KC, 128, 3 * E], bf16, isOutput=False)
    bqkv_d = nc.declare_dram_parameter("bqkv", [1, 3 * E], bf16, isOutput=False)
    wo_d = nc.declare_dram_parameter("wo", [KC, 128, E], bf16, isOutput=False)
    bo_d = nc.declare_dram_parameter("bo", [1, E], bf16, isOutput=False)
    y_d = nc.declare_dram_parameter("y", [TOK, E], f32, isOutput=True)

    with TileContext(nc) as tc:
        with (
            tc.tile_pool(name="wpool", bufs=1) as wp,
            tc.tile_pool(name="qkvpool", bufs=3) as qp,
            tc.tile_pool(name="xpool", bufs=3) as xp,
            tc.tile_pool(name="attnpool", bufs=4) as ap_,
            tc.tile_pool(name="opool", bufs=3) as op_,
            tc.tile_pool(name="prodpool", bufs=1) as pp,
            tc.tile_pool(name="ypool", bufs=2) as yp,
            tc.tile_pool(name="psqkv", bufs=4, space="PSUM") as ps_qkv,
            tc.tile_pool(name="pst", bufs=2, space="PSUM") as ps_t,
            tc.tile_pool(name="psy", bufs=2, space="PSUM") as ps_y,
        ):
            # ---- persistent weights; x streamed per-tile ----
            xt_t = {}

            def _load_xtile(i):
                xt = xp.tile([128, KC, PT], bf16, tag="xt")
                nc.sync.dma_start(
                    xt, xT_d[:, :, ts(i, PT)].rearrange("k p t -> p k t")
                )
                xt_t[i] = xt

            _load_xtile(0)
            wqkv_sb = wp.tile([128, KC, 3 * E], bf16)
            if os.environ.get("KERNEL_WLOAD", "1") == "2":
                # q,k columns as small per-(k-chunk, j-group) transfers so
                # phase1(0) starts as soon as the first 128KB lands
                for j in range(4):
                    for kk in range(KC):
                        nc.sync.dma_start(
                            wqkv_sb[:, kk, ds(j * 512, 512)],
                            wqkv_d[kk, :, ds(j * 512, 512)],
                        )
                for sl in range(8, 12):
                    nc.sync.dma_start(
                        wqkv_sb[:, :, ds(sl * 256, 256)],
                        wqkv_d[:, :, ds(sl * 256, 256)].rearrange("k p e -> p k e"),
                    )
            else:
                _WSPL = 12
                for sl in range(_WSPL):
                    w_ = 3 * E // _WSPL
                    nc.sync.dma_start(
                        wqkv_sb[:, :, ds(sl * w_, w_)],
                        wqkv_d[:, :, ds(sl * w_, w_)].rearrange("k p e -> p k e"),
                    )
            bq1 = wp.tile([1, 3 * E], bf16)
            nc.sync.dma_start(bq1, bqkv_d[:, :])
            bo1 = wp.tile([1, E], bf16)
            nc.sync.dma_start(bo1, bo_d[:, :])
            wo_sb = wp.tile([128, KC, E], bf16)
            nc.sync.dma_start(wo_sb, wo_d.rearrange("k p e -> p k e"))
            ones = wp.tile([1, 128], bf16)
            nc.vector.memset(ones, 1.0)
            ident = wp.tile([128, 128], bf16)
            make_identity(nc, ident)
            _WARM = int(os.environ.get("KERNEL_WARM", "96"))
            if _WARM > 0:
                # p-state warmup: junk transposes chained on the early x-tile
                # keep PE's ramp alive until the weights arrive
                psw = ps_t.tile([128, KC, 128], bf16, tag="pst")
                for wi in range(_WARM):
                    nc.tensor.transpose(
                        psw[:, wi % KC, :], xt_t[0][:, wi % KC, :], ident
                    )

            qkv_t, vt_t, ex_t, pav_t, oT_t = {}, {}, {}, {}, {}

            def phase1_part(i, js):
                xt, qkv = ph1_state[i]
                for j in js:
                    ps = ps_qkv.tile([128, 512], f32, tag="psq")
                    for k in range(KC):
                        nc.tensor.matmul(
                            ps,
                            xt[:, k, :],
                            wqkv_sb[:, k, ds(j * 512, 512)],
                            start=(k == 0),
                            stop=False,
                        )
                    nc.tensor.matmul(
                        ps, ones[0:1, :], bq1[0:1, ds(j * 512, 512)],
                        start=False, stop=True,
                    )
                    nc.scalar.copy(qkv[:, ds(j * 512, 512)], ps)

            ph1_state = {}

            def phase1(i, js=None):
                # qkv = x @ Wqkv + b   (token-major [tok, 3E])
                if i not in ph1_state:
                    xt = xt_t.pop(i)
                    qkv = qp.tile([128, 3 * E], bf16, tag="qkv")
                    ph1_state[i] = (xt, qkv)
                    qkv_t[i] = qkv
                phase1_part(i, range(6) if js is None else js)

            def scores_stage(i):
                qkv = qkv_t[i]
                k_ap = qkv[:, E : 2 * E].rearrange("p (u t d) -> p u t d", t=H, u=1)
                v_ap = qkv[:, 2 * E : 3 * E].rearrange("p (t d) -> p d t", t=H)
                vt = ap_.tile([128, DH, H], bf16, tag="vt")
                with tc.high_priority():
                    nc.scalar.copy(vt, v_ap)
                vt_t[i] = vt

                # Pool heads [0, PHSC)
                psc_g = None
                if _PHSC > 0:
                    hn = _PHSC
                    q_gp = qkv[:, 0 : hn * DH].rearrange(
                        "p (h u d) -> p h u d", h=hn, u=1
                    )
                    psc_g = pp.tile([128, hn, H, DH], bf16, tag="psc_gp")
                    nc.gpsimd.tensor_mul(
                        psc_g,
                        q_gp.broadcast_to([128, hn, H, DH]),
                        k_ap.broadcast_to([128, hn, H, DH]),
                    )
                    w = DH
                    while w > 1:
                        w //= 2
                        nc.gpsimd.tensor_add(
                            psc_g[:, :, :, 0:w], psc_g[:, :, :, 0:w],
                            psc_g[:, :, :, w : 2 * w],
                        )
                # DVE heads [PHSC, H)
                hn = H - _PHSC
                q_dv = qkv[:, ds(_PHSC * DH, hn * DH)].rearrange(
                    "p (h u d) -> p h u d", h=hn, u=1
                )
                psc = pp.tile([128, hn, H, DH], bf16, tag="psc_dve")
                nc.vector.tensor_mul(
                    psc,
                    q_dv.broadcast_to([128, hn, H, DH]),
                    k_ap.broadcast_to([128, hn, H, DH]),
                )
                w = DH
                while w > 1:
                    w //= 2
                    eng = (nc.gpsimd
                           if (w == 1 and os.environ.get("KERNEL_SCFL", "0") == "1")
                           else nc.vector)
                    eng.tensor_add(
                        psc[:, :, :, 0:w], psc[:, :, :, 0:w],
                        psc[:, :, :, w : 2 * w],
                    )
                # exp (no max-sub; |scores/32| is small). Both slices stay
                # unnormalized bf16; 1/Z is applied at extraction (next wave)
                # so no cross-engine edge lands inside this wave.
                scl = float(E) ** -0.5
                ex = ap_.tile([128, H, H], bf16, tag="ex")
                with tc.high_priority():
                    if _PHSC > 0:
                        nc.scalar.activation(
                            ex[:, 0:_PHSC, :], psc_g[:, :, :, 0], AF.Exp, scale=scl
                        )
                    nc.scalar.activation(
                        ex[:, ds(_PHSC, hn), :], psc[:, :, :, 0], AF.Exp, scale=scl
                    )
                ex_t[i] = ex

            def av_stage(i):
                ex = ex_t[i]
                vt_b = vt_t.pop(i).rearrange("p (u d) t -> p u d t", u=1)
                pav_g = None
                if _PHAV > 0:
                    hn = _PHAV
                    pav_g = pp.tile([128, hn, DH, H], bf16, tag="pav_gp")
                    a_sl = ex[:, 0:hn, :].rearrange("p h (u t) -> p h u t", u=1)
                    nc.gpsimd.tensor_mul(
                        pav_g,
                        a_sl.broadcast_to([128, hn, DH, H]),
                        vt_b.broadcast_to([128, hn, DH, H]),
                    )
                    w = H
                    while w > 1:
                        w //= 2
                        nc.gpsimd.tensor_add(
                            pav_g[:, :, :, 0:w], pav_g[:, :, :, 0:w],
                            pav_g[:, :, :, w : 2 * w],
                        )
                hn = H - _PHAV
                pav = pp.tile([128, hn, DH, H], bf16, tag="pav_dve")
                a_sl = ex[:, ds(_PHAV, hn), :].rearrange("p h (u t) -> p h u t", u=1)
                nc.vector.tensor_mul(
                    pav,
                    a_sl.broadcast_to([128, hn, DH, H]),
                    vt_b.broadcast_to([128, hn, DH, H]),
                )
                w = H
                while w > 1:
                    w //= 2
                    # the w==1 level has a count-1 last dim (1x on DVE); Pool
                    # absorbs it since the AV chain is not latency-critical
                    eng = nc.gpsimd if (w == 1 and _FLPOOL) else nc.vector
                    eng.tensor_add(
                        pav[:, :, :, 0:w], pav[:, :, :, 0:w],
                        pav[:, :, :, w : 2 * w],
                    )
                pav_t[i] = (pav_g, pav)

            def transpose_stage(i):
                ex = ex_t.pop(i)
                pav_g, pav = pav_t.pop(i)
                zr = ap_.tile([128, H], f32, tag="zr")
                nc.vector.reduce_sum(zr, ex, axis=mybir.AxisListType.X)
                nc.vector.reciprocal(zr, zr)
                o_c = op_.tile([128, E], bf16, tag="oc")
                with tc.high_priority():
                    for hh in range(H):
                        pv = (pav_g[:, hh, :, 0] if hh < _PHAV
                              else pav[:, hh - _PHAV, :, 0])
                        nc.scalar.activation(
                            o_c[:, ds(hh * DH, DH)], pv, AF.Copy,
                            scale=zr[:, hh : hh + 1],
                        )
                pst = ps_t.tile([128, KC, 128], bf16, tag="pst")
                oT = op_.tile([128, KC, 128], bf16, tag="oT")
                for m in range(KC):
                    nc.tensor.transpose(pst[:, m, :], o_c[:, ts(m, 128)], ident)
                with tc.high_priority():
                    nc.scalar.copy(oT[:, 0:4, :], pst[:, 0:4, :])
                    nc.scalar.copy(oT[:, 4:8, :], pst[:, 4:8, :])
                oT_t[i] = oT

            def oproj_stage(i):
                oT = oT_t.pop(i)
                ysb = yp.tile([128, E], f32, tag="ysb")
                for j in range(2):
                    psy = ps_y.tile([128, 512], f32, tag="psy")
                    for m in range(KC):
                        nc.tensor.matmul(
                            psy, oT[:, m, :], wo_sb[:, m, ds(j * 512, 512)],
                            start=(m == 0), stop=False,
                        )
                    nc.tensor.matmul(
                        psy, ones[0:1, :], bo1[0:1, ds(j * 512, 512)],
                        start=False, stop=True,
                    )
                    if (i == NT - 1 and j == 1
                            and os.environ.get("KERNEL_YDVE", "0") == "1"):
                        nc.vector.tensor_copy(ysb[:, ds(j * 512, 512)], psy)
                    else:
                        nc.scalar.copy(ysb[:, ds(j * 512, 512)], psy)
                _ysp = int(os.environ.get("KERNEL_YSPL", "2"))
                for q in range(_ysp):
                    w_ = E // _ysp
                    nc.sync.dma_start(
                        y_d[ts(i, PT), ds(q * w_, w_)], ysb[:, ds(q * w_, w_)]
                    )

            for w in range(NT + _LAG_T):
                if w + 1 < NT:
                    _load_xtile(w + 1)
                _sc1 = os.environ.get("KERNEL_SCE", "2")
                if (w == 2 and os.environ.get("KERNEL_AV2", "0") == "1"
                        and 2 <= w <= NT + 1):
                    av_stage(0)
                if _sc1 == "2" and 1 <= w <= NT:
                    scores_stage(w - 1)
                if _sc1 == "1" and w == 1:
                    scores_stage(0)
                if w < NT:
                    phase1(w)
                if 2 <= w <= NT + 1 and not (
                        w == 2 and os.environ.get("KERNEL_AV2", "0") == "1"):
                    av_stage(w - 2)
                if w >= _LAG_T:
                    transpose_stage(w - _LAG_T)
                if (_sc1 == "0" and 1 <= w <= NT) or (_sc1 == "1" and 2 <= w <= NT):
                    scores_stage(w - 1)
                if w >= _LAG_T:
                    oproj_stage(w - _LAG_T)

    _legalize_waits(nc, mybir)
    return nc


def _legalize_waits(nc, mybir):
    """This walrus build allows only ONE sync wait per engine instruction.
    Split extra waits into standalone same-engine EventSemaphore insts."""
    for f in nc.m.functions:
        for b in f.blocks:
            newl = []
            for inst in b.instructions:
                si = getattr(inst, "sync_info", None)
                ow = list(si.on_wait) if si and si.on_wait else []
                if len(ow) > 1:
                    for w in ow[:-1]:
                        newl.append(
                            mybir.InstEventSemaphore(
                                name=f"WS-{nc.next_id()}",
                                engine=inst.engine,
                                sync_info=mybir.SyncInfo(on_wait=[w], on_update=[]),
                            )
                        )
                    si.on_wait = [ow[-1]]
                newl.append(inst)
            b.instructions = newl


def _prep_weights(w_qkv, b_qkv, w_o, b_o):
    # permute fused-qkv columns: orig e = h*192 + part*64 + d
    #                           new  e = part*1024 + h*64 + d
    part, h, d = np.meshgrid(
        np.arange(3), np.arange(H), np.arange(DH), indexing="ij"
    )
    perm = (h * 192 + part * 64 + d).reshape(-1)
    wq = np.ascontiguousarray(w_qkv[:, perm]).astype(ml_dtypes.bfloat16)
    bq = np.ascontiguousarray(b_qkv[perm]).astype(ml_dtypes.bfloat16)[None, :]
    wo = np.ascontiguousarray(w_o).astype(ml_dtypes.bfloat16)
    return (
        wq.reshape(KC, 128, 3 * E),
        bq,
        wo.reshape(KC, 128, E),
        np.asarray(b_o).astype(ml_dtypes.bfloat16)[None, :],
    )


def kernel(x, w_qkv, b_qkv, w_o, b_o):
    from concourse.bass_utils import run_bass_kernel_spmd

    if "nc" not in _CACHE:
        _CACHE["nc"] = _build_nc()
    nc = _CACHE["nc"]

    wq, bq, wo, bo = _prep_weights(
        np.asarray(w_qkv, np.float32),
        np.asarray(b_qkv, np.float32),
        np.asarray(w_o, np.float32),
        np.asarray(b_o, np.float32),
    )
    x = np.asarray(x, np.float32)
    in_maps = []
    for c in range(NCORES):
        xc = x[2 * c : 2 * c + 2].reshape(TOK, C)
        xT = np.ascontiguousarray(xc.T).astype(ml_dtypes.bfloat16)
        in_maps.append(
            {
                "xT": xT.reshape(KC, 128, TOK),
                "wqkv": wq,
                "bqkv": bq,
                "wo": wo,
                "bo": bo,
            }
        )

    res = run_bass_kernel_spmd(nc, in_maps, core_ids=list(range(NCORES)))
    out = np.empty((B, S, E), np.float32)
    for c in range(NCORES):
        out[2 * c : 2 * c + 2] = res.results[c]["y"].reshape(2, S, E)
    return out
